# revision 1
# baseline (speedup 1.0000x reference)
"""Trainium2 Bass kernel for the CGF tree-GRU problem.

Problem: 3-level complete 8-ary tree GRU (torch GRU cell convention).
  Level 3: 64 nodes x 8 embedded leaf children, h0 = 0
  Level 2:  8 nodes x 8 children (level-3 outputs), h0 = mean of children h
  Level 1:  1 node  x 8 children (level-2 outputs), h0 = mean of children h
  Output: mean over the 8 step outputs of the root GRU. D = 512.

Distribution choice: the computation is ONE serial chain of 24 GRU steps
(8 per level; levels strictly dependent).  Each step is dominated by moving
W_hh (1536x512) through the PE array, independent of the node-batch size, so
sharding the node batch across cores saves nothing, and sharding the hidden
dim requires a per-step all-gather whose latency exceeds a whole step.  The
kernel is therefore replicated on all 8 cores (SPMD, identical inputs); core
0's output is returned.

Layout: everything lives TRANSPOSED on chip - gate/hidden dims on the 128
partitions (4 or 12 tiles of 128), batch on the free dim.  This makes GRU
biases per-partition scalars, halves DVE cost vs. the natural layout, and
removes all transposes: the recurrent matmul gh^T = W_hh @ h^T consumes h^T
directly, and each level's mean-output feeds the next level's input matmul
without reshaping.

Precision: matmul operands are bf16 (PSUM accumulation is fp32); everything
else - the carried state h, gi, gates, biases - stays fp32.  A bf16 shadow
of h feeds the matmuls: storing the state itself in bf16 costs ~8e-3
scale-relative error while bf16 matmul inputs only cost ~1e-3 (measured).

Scheduling shape per step: the 48 W_hh matmuls are a pure LDW-rate-bound
burst (r,z gate slices first) into four single-bank PSUM tiles, so the
chunked gi+gh adds / sigmoids stream on DVE/ACT behind the burst without
tripping same-bank PE-write/DVE-read serialization.  The fp32 state update
and the output accumulation run during the NEXT step's burst on the
otherwise-idle gpsimd engine - only the bf16 shadow write gates the next
matmul burst.
"""

import numpy as np

import concourse.bacc as bacc
import concourse.mybir as mybir
from concourse.tile import TileContext
from concourse.bass_utils import run_bass_kernel_spmd

AF = mybir.ActivationFunctionType
OP = mybir.AluOpType
FP = mybir.dt.float32
BF = mybir.dt.bfloat16

P = 128          # partitions
D = 512          # hidden size
KT = D // P      # 4 k-tiles (contraction)
G = 3 * D        # 1536 gate dims
MT = G // P      # 12 m-tiles (gate rows)
A = 8            # tree arity == sequence length per level
NB = 64          # level-3 node count
T = 8            # steps per level
N_CORES = 8

# bf16 blob layout: [xt(2048) | wit(6144) | wht(6144)]
O_XT = 0
O_WIT = O_XT + KT * T * NB
O_WHT = O_WIT + MT * KT * P
B16_COLS = O_WHT + MT * KT * P
# fp32 blob: [gb(12) | bhn(4) | bhnb(256)]
B32_COLS = MT + KT + KT * NB

_BUILT = None  # cached Bass module


def _v(ap, g):
    """View a 2-D [P, g*b] AP as [P, g, b]."""
    return ap.rearrange("p (g b) -> p g b", g=g)


def _build_nc():
    nc = bacc.Bacc()

    blob16 = nc.declare_dram_parameter("blob16", [P, B16_COLS], BF, isOutput=False)
    blob32 = nc.declare_dram_parameter("blob32", [P, B32_COLS], FP, isOutput=False)
    outp = nc.declare_dram_parameter("out", [P, KT], FP, isOutput=True)

    with TileContext(nc) as tc:
        with (
            tc.tile_pool(name="const", bufs=1) as cpool,
            tc.tile_pool(name="state", bufs=1) as spool,
            tc.tile_pool(name="work", bufs=2) as wpool,
            tc.tile_pool(name="pg", bufs=4, space="PSUM") as gpool,
            tc.tile_pool(name="prza", bufs=1, space="PSUM") as rzapool,
            tc.tile_pool(name="przb", bufs=1, space="PSUM") as rzbpool,
            tc.tile_pool(name="pna", bufs=1, space="PSUM") as napool,
            tc.tile_pool(name="pnb", bufs=1, space="PSUM") as nbpool,
        ):
            # Warm the activation tables before anything else: the lazy
            # ACT_TABLE_LOADs otherwise land mid-kernel and stall the first
            # sigmoid/tanh by >1us each.
            warm = cpool.tile([P, 8], FP)
            nc.vector.memset(warm[:, :], 0.0)
            for fn in (AF.Identity, AF.Sigmoid, AF.Tanh):
                nc.scalar.activation(warm[:, :], warm[:, :], fn)

            # Chunked input DMA (a wide DMA fans out over many HW-DGE queues
            # and blows the per-instruction sync-wait budget downstream; all
            # consumer slices stay within one 1024-col chunk).  Chunks
            # alternate between the two HWDGE-capable engines' rings so two
            # transfers are in flight at once.
            b32_sb = cpool.tile([P, B32_COLS], FP)
            nc.scalar.dma_start(out=b32_sb[:], in_=blob32[:, :])
            b16_sb = cpool.tile([P, B16_COLS], BF)
            for i, c0 in enumerate(range(0, B16_COLS, 1024)):
                c1 = min(c0 + 1024, B16_COLS)
                eng = nc.sync if i % 2 == 0 else nc.scalar
                eng.dma_start(out=b16_sb[:, c0:c1], in_=blob16[:, c0:c1])

            xt_sb = b16_sb[:, O_XT : O_XT + KT * T * NB]
            wit_sb = b16_sb[:, O_WIT : O_WIT + MT * KT * P]
            wht_sb = b16_sb[:, O_WHT : O_WHT + MT * KT * P]
            gb_sb = b32_sb[:, 0:MT]
            bhn_sb = b32_sb[:, MT : MT + KT]
            bhnb_sb = b32_sb[:, MT + KT : MT + KT + KT * NB]

            def compute_gi(gi_tile, rhs_of_k, ncols):
                """gi^T = W_ih @ x^T + combined bias (fp32 out), m-major."""
                for m in range(MT):
                    ps = gpool.tile([P, ncols], FP, tag="gi_ps")
                    for k in range(KT):
                        nc.tensor.matmul(
                            ps[:, :],
                            lhsT=wit_sb[:, (m * KT + k) * P : (m * KT + k + 1) * P],
                            rhs=rhs_of_k(k),
                            start=(k == 0),
                            stop=(k == KT - 1),
                        )
                    nc.scalar.activation(
                        gi_tile[:, m * ncols : (m + 1) * ncols],
                        ps[:, :],
                        AF.Identity,
                        bias=gb_sb[:, m : m + 1],
                        scale=1.0,
                    )

            def gru_level(B, h_tile, h16_tile, acc_tile, gi_rz_at, gi_n_at, zero_h0):
                """8 GRU steps.  h_tile [P, KT*B] fp32 state, h16_tile bf16
                shadow feeding the matmuls, acc_tile fp32 output accumulator.
                gi_rz_at(t) -> [P, 8, B] AP, gi_n_at(t) -> [P, 4, B] AP."""
                for t in range(T):
                    if t == 0 and zero_h0:
                        # h = 0 so gh == b_hh exactly; skip the matmuls.
                        rzt = wpool.tile([P, 8 * B], FP, tag="rz")
                        nc.scalar.activation(_v(rzt[:], 8), gi_rz_at(t), AF.Sigmoid)
                        bt = wpool.tile([P, KT * B], FP, tag="bt")
                        nc.vector.tensor_mul(
                            _v(bt[:], KT),
                            _v(rzt[:, : KT * B], KT),
                            _v(bhnb_sb, KT)[:, :, :B],
                        )
                        ct = wpool.tile([P, KT * B], FP, tag="ct")
                        nc.vector.tensor_add(_v(ct[:], KT), _v(bt[:], KT), gi_n_at(t))
                        nt = wpool.tile([P, KT * B], FP, tag="nt")
                        nc.scalar.activation(nt[:, :], ct[:, :], AF.Tanh)
                        # h1 = (1 - z) * n = n - z*n
                        ft = wpool.tile([P, KT * B], FP, tag="ft")
                        nc.vector.tensor_mul(ft[:, :], rzt[:, KT * B :], nt[:, :])
                        nc.vector.tensor_sub(h16_tile[:, :], nt[:, :], ft[:, :])
                        nc.vector.tensor_sub(h_tile[:, :], nt[:, :], ft[:, :])
                        nc.vector.tensor_copy(acc_tile[:, :], h_tile[:, :])
                        continue

                    # One PSUM bank per quarter so the streaming DVE/ACT
                    # reads never touch a bank the PE is still writing
                    # (same-bank PE-write / DVE-read pairs get serialized).
                    ps_rza = rzapool.tile([P, 4 * B], FP, tag="ps_rza")
                    ps_rzb = rzbpool.tile([P, 4 * B], FP, tag="ps_rzb")
                    ps_na = napool.tile([P, 2 * B], FP, tag="ps_na")
                    ps_nb = nbpool.tile([P, 2 * B], FP, tag="ps_nb")
                    arz = wpool.tile([P, 8 * B], FP, tag="arz")
                    rzt = wpool.tile([P, 8 * B], FP, tag="rz")
                    rb = wpool.tile([P, KT * B], FP, tag="rb")
                    bt = wpool.tile([P, KT * B], FP, tag="bt")
                    ct = wpool.tile([P, KT * B], FP, tag="ct")
                    nt = wpool.tile([P, KT * B], FP, tag="nt")
                    gi_rz = gi_rz_at(t)
                    gi_n = gi_n_at(t)
                    # r,z slices first; gi+gh adds and sigmoids stream behind
                    # the burst, chunk by chunk, as their banks complete.
                    for m in range(MT):
                        if m < 4:
                            dst = ps_rza[:, m * B : (m + 1) * B]
                        elif m < 8:
                            dst = ps_rzb[:, (m - 4) * B : (m - 3) * B]
                        elif m < 10:
                            dst = ps_na[:, (m - 8) * B : (m - 7) * B]
                        else:
                            dst = ps_nb[:, (m - 10) * B : (m - 9) * B]
                        for k in range(KT):
                            nc.tensor.matmul(
                                dst,
                                lhsT=wht_sb[:, (m * KT + k) * P : (m * KT + k + 1) * P],
                                rhs=h16_tile[:, k * B : (k + 1) * B],
                                start=(k == 0),
                                stop=(k == KT - 1),
                            )
                        if m == 3 or m == 7:
                            lo = 0 if m == 3 else 4
                            src = ps_rza if m == 3 else ps_rzb
                            nc.vector.tensor_add(
                                _v(arz[:, lo * B : (lo + 4) * B], 4),
                                _v(src[:], 4),
                                gi_rz[:, lo : lo + 4],
                            )
                            nc.scalar.activation(
                                rzt[:, lo * B : (lo + 4) * B],
                                arz[:, lo * B : (lo + 4) * B],
                                AF.Sigmoid,
                            )
                            if m == 7:
                                # rb = r*b_hn + gi_n (hidden under the n-gate
                                # matmuls); b_hn as a broadcast tensor keeps
                                # it to two full-width ops.
                                nc.vector.tensor_mul(
                                    _v(bt[:], KT),
                                    _v(rzt[:, : KT * B], KT),
                                    _v(bhnb_sb, KT)[:, :, :B],
                                )
                                nc.vector.tensor_add(
                                    _v(rb[:], KT), _v(bt[:], KT), gi_n
                                )
                    # b = gh_n * r + rb;  n = tanh(b)
                    bn = wpool.tile([P, KT * B], FP, tag="bn")
                    nc.vector.tensor_mul(
                        bn[:, 0 : 2 * B], ps_na[:], rzt[:, 0 : 2 * B]
                    )
                    nc.vector.tensor_mul(
                        bn[:, 2 * B : KT * B], ps_nb[:], rzt[:, 2 * B : KT * B]
                    )
                    nc.vector.tensor_add(ct[:, :], bn[:, :], rb[:, :])
                    nc.scalar.activation(nt[:, :], ct[:, :], AF.Tanh)
                    # h' = n + z*(h - n); bf16 shadow written first (it gates
                    # the next burst), fp32 state + acc follow during it on
                    # the otherwise-idle gpsimd engine.
                    et = wpool.tile([P, KT * B], FP, tag="et")
                    nc.vector.tensor_sub(et[:, :], h_tile[:, :], nt[:, :])
                    ft = wpool.tile([P, KT * B], FP, tag="ft")
                    nc.vector.tensor_mul(ft[:, :], rzt[:, KT * B :], et[:, :])
                    nc.vector.tensor_add(h16_tile[:, :], nt[:, :], ft[:, :])
                    slow_eng = nc.gpsimd if t < T - 1 else nc.vector
                    slow_eng.tensor_add(h_tile[:, :], nt[:, :], ft[:, :])
                    if t == 0:
                        slow_eng.tensor_copy(acc_tile[:, :], h_tile[:, :])
                    else:
                        slow_eng.tensor_add(
                            acc_tile[:, :], acc_tile[:, :], h_tile[:, :]
                        )

            # ---------------- Level 3: 64 nodes ----------------
            gi3 = cpool.tile([P, MT * T * NB], FP)
            compute_gi(gi3, lambda k: xt_sb[:, k * (T * NB) : (k + 1) * (T * NB)], T * NB)
            gi3v = gi3[:].rearrange("p (m t b) -> p m t b", m=MT, t=T)
            h3 = spool.tile([P, KT * NB], FP)
            h3s = spool.tile([P, KT * NB], BF)
            acc3 = spool.tile([P, KT * NB], FP)
            gru_level(
                NB, h3, h3s, acc3,
                lambda t: gi3v[:, 0:8, t],
                lambda t: gi3v[:, 8:12, t],
                zero_h0=True,
            )

            # ---------------- Level 3 -> 2 transition ----------------
            x2 = spool.tile([P, KT * NB], BF)
            nc.scalar.mul(x2[:, :], acc3[:, :], 1.0 / A)
            hr2 = spool.tile([P, KT * A], FP)
            nc.vector.tensor_reduce(
                _v(hr2[:], KT),
                h3[:].rearrange("p (k j c) -> p k j c", k=KT, j=A),
                axis=mybir.AxisListType.X,
                op=OP.add,
            )
            h2 = spool.tile([P, KT * A], FP)
            nc.scalar.mul(h2[:, :], hr2[:, :], 1.0 / A)
            h2s = spool.tile([P, KT * A], BF)
            nc.scalar.mul(h2s[:, :], hr2[:, :], 1.0 / A)

            gi2 = cpool.tile([P, MT * NB], FP)
            compute_gi(gi2, lambda k: x2[:, k * NB : (k + 1) * NB], NB)
            # gi2 within-m column order is (j, t); step-t slices are strided.
            gi2v = gi2[:].rearrange("p (m j t) -> p m j t", m=MT, j=A)
            acc2 = spool.tile([P, KT * A], FP)
            gru_level(
                A, h2, h2s, acc2,
                lambda t: gi2v[:, 0:8, :, t],
                lambda t: gi2v[:, 8:12, :, t],
                zero_h0=False,
            )

            # ---------------- Level 2 -> 1 transition ----------------
            x1 = spool.tile([P, KT * A], BF)
            nc.scalar.mul(x1[:, :], acc2[:, :], 1.0 / A)
            hr1 = spool.tile([P, KT], FP)
            nc.vector.tensor_reduce(
                _v(hr1[:], KT),
                h2[:].rearrange("p (k j c) -> p k j c", k=KT, j=1),
                axis=mybir.AxisListType.X,
                op=OP.add,
            )
            h1 = spool.tile([P, KT], FP)
            nc.scalar.mul(h1[:, :], hr1[:, :], 1.0 / A)
            h1s = spool.tile([P, KT], BF)
            nc.scalar.mul(h1s[:, :], hr1[:, :], 1.0 / A)

            gi1 = cpool.tile([P, MT * A], FP)
            compute_gi(gi1, lambda k: x1[:, k * A : (k + 1) * A], A)
            gi1v = gi1[:].rearrange("p (m t) -> p m t", m=MT)
            acc1 = spool.tile([P, KT], FP)
            gru_level(
                1, h1, h1s, acc1,
                lambda t: gi1v[:, 0:8, t : t + 1],
                lambda t: gi1v[:, 8:12, t : t + 1],
                zero_h0=False,
            )

            out_sb = spool.tile([P, KT], FP)
            nc.scalar.mul(out_sb[:, :], acc1[:, :], 1.0 / A)
            nc.sync.dma_start(out=outp[:, :], in_=out_sb[:, :])

    nc.finalize()
    return nc


def _get_nc():
    global _BUILT
    if _BUILT is None:
        _BUILT = _build_nc()
    return _BUILT


def make_inputs(leaf_ids, embed_table, W_ih, W_hh, b_ih, b_hh):
    """Host-side shard/layout prep: gather the looked-up embedding rows and
    lay every tensor out in the on-chip transposed format."""
    import ml_dtypes

    leaf_ids = np.asarray(leaf_ids).astype(np.int64)
    emb = np.asarray(embed_table, dtype=np.float32)
    W_ih = np.asarray(W_ih, dtype=np.float32)
    W_hh = np.asarray(W_hh, dtype=np.float32)
    b_ih = np.asarray(b_ih, dtype=np.float32)
    b_hh = np.asarray(b_hh, dtype=np.float32)

    x = emb[leaf_ids]  # [64, 8, 512]
    # time-major batch: row b = t*64 + node
    xtm = np.ascontiguousarray(x.transpose(1, 0, 2)).reshape(T * NB, D)
    xt_in = np.ascontiguousarray(
        xtm.T.reshape(KT, P, T * NB).transpose(1, 0, 2)
    ).reshape(P, KT * T * NB)

    def pack_w(W):  # W [1536, 512] -> lhsT tiles [(m,k) major]
        WT = np.ascontiguousarray(W.T)  # [512, 1536]
        return np.ascontiguousarray(
            WT.reshape(KT, P, MT, P).transpose(1, 2, 0, 3)
        ).reshape(P, MT * KT * P)

    blob16 = np.concatenate([xt_in, pack_w(W_ih), pack_w(W_hh)], axis=1).astype(
        ml_dtypes.bfloat16
    )

    gbias = np.concatenate([(b_ih + b_hh)[: 2 * D], b_ih[2 * D :]])
    gb_in = np.ascontiguousarray(gbias.reshape(MT, P).T)
    bhn_in = np.ascontiguousarray(b_hh[2 * D :].reshape(KT, P).T)
    bhnb_in = np.ascontiguousarray(np.repeat(bhn_in, NB, axis=1))
    blob32 = np.concatenate([gb_in, bhn_in, bhnb_in], axis=1)

    assert blob16.shape == (P, B16_COLS) and blob32.shape == (P, B32_COLS)
    return {
        "blob16": np.ascontiguousarray(blob16),
        "blob32": np.ascontiguousarray(blob32),
    }


def unpack_output(out_np):
    # out [P, KT]: element (p, k) = root dim k*128+p
    return np.ascontiguousarray(out_np.T).reshape(1, 1, D).astype(np.float32)


def kernel(leaf_ids=None, layer=None, embed_table=None, W_ih=None, W_hh=None,
           b_ih=None, b_hh=None, **_unused):
    in_map = make_inputs(leaf_ids, embed_table, W_ih, W_hh, b_ih, b_hh)
    nc = _get_nc()
    res = run_bass_kernel_spmd(nc, [in_map] * N_CORES, list(range(N_CORES)))
    return unpack_output(res.results[0]["out"])



# revision 8
# speedup vs baseline: 1.1428x; 1.1428x over previous
"""Trainium2 Bass kernel for the CGF tree-GRU problem.

Problem: 3-level complete 8-ary tree GRU (torch GRU cell convention).
  Level 3: 64 nodes x 8 embedded leaf children, h0 = 0
  Level 2:  8 nodes x 8 children (level-3 outputs), h0 = mean of children h
  Level 1:  1 node  x 8 children (level-2 outputs), h0 = mean of children h
  Output: mean over the 8 step outputs of the root GRU. D = 512.

Distribution: the computation is ONE serial chain of 24 GRU steps; each step
is LDW-rate-bound on the PE (W_hh streamed per step), so batch sharding
saves nothing and hidden-dim sharding needs a per-step collective that costs
more than a step.  Replicated SPMD on all 8 cores; core 0's output returned.

Performance design (v2): the PE array's HAM clock gate runs the PE at
1.2 GHz unless it has been busy ~continuously for ~3.4us (then 2.4 GHz).
The v1 kernel idled the PE 2-3us per step waiting on the eltwise chain and
ran cold (50% clock) 73% of the time.  v2 keeps the PE saturated:

- Per-step PSUM banks accumulate bias + W_ih@x + W_hh@h directly: an
  identity matmul preloads the (broadcast) bias with start=True, the
  level-3 per-step gi matmuls and the recurrent gh matmuls accumulate on
  top.  The sigmoid/tanh then read finished pre-activations straight from
  PSUM - no gi adds on the critical path.
- The next step's bias+gi "prep" matmuls are issued between recurrent
  bursts, so the PE computes useful work while the eltwise tail runs.
- Gate phase order r -> n -> z puts only sigmoid(z), ft, h16 after the
  last matmul of a burst.
- State h is bf16 (shadow == state); the output accumulator stays fp32.
- Levels 2/1 batch their gi into two temp banks at the level transition
  (amortizes LDW over all 8 steps) and fill the eltwise-tail PE gap with
  dummy matmuls to keep the HAM gate warm.
"""

import numpy as np

import concourse.bacc as bacc
import concourse.mybir as mybir
from concourse.tile import TileContext
from concourse.bass_utils import run_bass_kernel_spmd

AF = mybir.ActivationFunctionType
OP = mybir.AluOpType
FP = mybir.dt.float32
BF = mybir.dt.bfloat16

P = 128          # partitions
D = 512          # hidden size
KT = D // P      # 4 k-tiles (contraction)
G = 3 * D        # 1536 gate dims
MT = G // P      # 12 m-tiles (gate rows)
A = 8            # tree arity == sequence length per level
NB = 64          # level-3 node count
T = 8            # steps per level
N_CORES = 8

# blob16 layout (columns, bf16):
O_ID = 0                      # identity [P,128]
O_BR = O_ID + P               # r-bias bcast  [P,4,64]
O_BZ = O_BR + 4 * NB          # z-bias bcast  [P,4,64]
O_BG = O_BZ + 4 * NB          # b_in bcast    [P,4,64]
O_BN = O_BG + 4 * NB          # b_hn bcast    [P,4,64]
O_BN8 = O_BN + 4 * NB         # b_hn bcast    [P,4,8]
O_BN1 = O_BN8 + 4 * A         # b_hn bcast    [P,4,1]
O_BB2 = O_BN1 + 4             # drain bias    [P,8,12,8]
O_BB1 = O_BB2 + T * MT * A    # drain bias    [P,8,12,1]
O_XT = O_BB1 + T * MT         # leaf embeds   [P,t,k,node]
O_WIT = O_XT + T * KT * NB    # W_ih tiles    [(m,k) major]
O_WHT = O_WIT + MT * KT * P   # W_hh tiles
B16_COLS = O_WHT + MT * KT * P

N_DUMMY = {2: 30, 1: 28}      # HAM-filler matmuls per step for levels 2/1

_BUILT = None


def _build_nc():
    nc = bacc.Bacc()

    blob16 = nc.declare_dram_parameter("blob16", [P, B16_COLS], BF, isOutput=False)
    outp = nc.declare_dram_parameter("out", [P, KT], FP, isOutput=True)

    with TileContext(nc) as tc:
        with (
            tc.tile_pool(name="const", bufs=1) as cpool,
            tc.tile_pool(name="state", bufs=1) as spool,
            tc.tile_pool(name="work", bufs=2) as wpool,
            tc.tile_pool(name="pr0", bufs=1, space="PSUM") as pr0,
            tc.tile_pool(name="pr1", bufs=1, space="PSUM") as pr1,
            tc.tile_pool(name="pz0", bufs=1, space="PSUM") as pz0,
            tc.tile_pool(name="pz1", bufs=1, space="PSUM") as pz1,
            tc.tile_pool(name="pn0", bufs=1, space="PSUM") as pn0,
            tc.tile_pool(name="pn1", bufs=1, space="PSUM") as pn1,
            tc.tile_pool(name="pg0", bufs=1, space="PSUM") as pg0,
            tc.tile_pool(name="pg1", bufs=1, space="PSUM") as pg1,
        ):
            # Warm the activation tables first (lazy ACT_TABLE_LOADs
            # otherwise land mid-kernel and stall sigmoids by >1us).
            warm = cpool.tile([P, 8], FP)
            nc.vector.memset(warm[:, :], 0.0)
            for fn in (AF.Identity, AF.Sigmoid, AF.Tanh):
                nc.scalar.activation(warm[:, :], warm[:, :], fn)

            # Chunked input DMA, ordered by first use, alternating between
            # two HWDGE-capable engines' rings.  Consumers slice regions of
            # b16_sb and gate on the chunks that cover them.
            b16_sb = cpool.tile([P, B16_COLS], BF)
            ranges = []
            CH = 512
            misc_end = O_XT
            for c in range(0, misc_end, CH):
                ranges.append((c, min(c + CH, misc_end)))
            ranges.append((O_XT, O_XT + CH))                 # xt t0-t1
            for c in range(O_WIT, O_WHT, CH):                # wit
                ranges.append((c, min(c + CH, O_WHT)))
            for c in range(O_WHT, B16_COLS, CH):             # wht
                ranges.append((c, min(c + CH, B16_COLS)))
            for c in range(O_XT + CH, O_WIT, CH):            # xt rest
                ranges.append((c, min(c + CH, O_WIT)))
            for i, (c0, c1) in enumerate(ranges):
                eng = nc.sync if i % 2 == 0 else nc.gpsimd
                eng.dma_start(out=b16_sb[:, c0:c1], in_=blob16[:, c0:c1])

            ident = b16_sb[:, O_ID:O_ID + P]
            br64 = b16_sb[:, O_BR:O_BR + 4 * NB]
            bz64 = b16_sb[:, O_BZ:O_BZ + 4 * NB]
            bg64 = b16_sb[:, O_BG:O_BG + 4 * NB]
            bn64 = b16_sb[:, O_BN:O_BN + 4 * NB]
            bn8 = b16_sb[:, O_BN8:O_BN8 + 4 * A]
            bn1 = b16_sb[:, O_BN1:O_BN1 + 4]
            bb2 = b16_sb[:, O_BB2:O_BB2 + T * MT * A]
            bb1 = b16_sb[:, O_BB1:O_BB1 + T * MT]
            xt = b16_sb[:, O_XT:O_XT + T * KT * NB]

            def wit(m, k):
                c = O_WIT + (m * KT + k) * P
                return b16_sb[:, c:c + P]

            def wht(m, k):
                c = O_WHT + (m * KT + k) * P
                return b16_sb[:, c:c + P]

            def mm(dst, lhsT, rhs, start, stop):
                nc.tensor.matmul(dst, lhsT=lhsT, rhs=rhs, start=start,
                                 stop=stop, skip_group_check=True)

            # ---------------- generic level runner ----------------
            def run_level(lvl, B, h0_tile, x_rhs_of_tk, gin_sb_of_t,
                          bias_r, bias_z, bias_n):
                """Runs 8 GRU steps for one level.

                lvl: 3, 2, or 1.  B: node batch.  h0_tile: [P, KT*B] bf16
                initial state (None for level 3 => zeros, no t0 gh burst).
                x_rhs_of_tk(t, k): rhs AP for the per-step gi matmuls
                (level 3 only).  gin_sb_of_t(t): SBUF AP [P, 4*B] holding
                gi_n + b_in (levels 2/1; level 3 passes None and uses the
                G PSUM bank).  Returns (h_final, acc) tiles.
                """
                rp = (pr0, pr1)
                zp = (pz0, pz1)
                np_ = (pn0, pn1)
                gp = (pg0, pg1)
                sfx = f"L{lvl}"
                hA = spool.tile([P, KT * B], BF, tag=f"hA{sfx}")
                hB = spool.tile([P, KT * B], BF, tag=f"hB{sfx}")
                acc = spool.tile([P, KT * B], FP, tag=f"acc{sfx}")
                if lvl == 3:
                    hz = spool.tile([P, KT * B], BF, tag=f"hz{sfx}")
                    nc.vector.memset(hz[:, :], 0.0)

                banks = {}

                def get_banks(t):
                    par = t & 1
                    key = (par,)
                    if key not in banks or banks[key][0] != t:
                        pR = rp[par].tile([P, 512], FP, tag=f"r{par}",
                                          name=f"pR{par}{sfx}")
                        pZ = zp[par].tile([P, 512], FP, tag=f"z{par}",
                                          name=f"pZ{par}{sfx}")
                        pN = np_[par].tile([P, 512], FP, tag=f"n{par}",
                                           name=f"pN{par}{sfx}")
                        pG = (gp[par].tile([P, 512], FP, tag=f"g{par}",
                                           name=f"pG{par}{sfx}")
                              if lvl == 3 else None)
                        banks[key] = (t, pR, pZ, pN, pG)
                    return banks[key][1:]

                def emit_prep(t, final):
                    pR, pZ, pN, pG = get_banks(t)
                    if lvl == 3:
                        mm(pR[:, :4 * B], ident, bias_r, start=True,
                           stop=False)
                        for m in range(4):
                            for k in range(KT):
                                mm(pR[:, m * B:(m + 1) * B], wit(m, k),
                                   x_rhs_of_tk(t, k), start=False,
                                   stop=final and m == 3 and k == KT - 1)
                        mm(pZ[:, :4 * B], ident, bias_z, start=True,
                           stop=False)
                        for m in range(4, 8):
                            for k in range(KT):
                                mm(pZ[:, (m - 4) * B:(m - 3) * B], wit(m, k),
                                   x_rhs_of_tk(t, k), start=False,
                                   stop=final and m == 7 and k == KT - 1)
                        mm(pG[:, :4 * B], ident, bias_n, start=True,
                           stop=False)
                        for m in range(8, 12):
                            for k in range(KT):
                                mm(pG[:, (m - 8) * B:(m - 7) * B], wit(m, k),
                                   x_rhs_of_tk(t, k), start=False,
                                   stop=m == 11 and k == KT - 1)
                        mm(pN[:, :4 * B], ident, bn64[:, :4 * B],
                           start=True, stop=final)
                    else:
                        gsb = gin_sb_of_t(t)
                        mm(pR[:, :4 * B], ident, gsb[:, 0:4 * B],
                           start=True, stop=False)
                        mm(pZ[:, :4 * B], ident, gsb[:, 4 * B:8 * B],
                           start=True, stop=False)
                        mm(pN[:, :4 * B], ident,
                           (bn8 if lvl == 2 else bn1)[:, :4 * B],
                           start=True, stop=False)

                def emit_gh(t, h_prev):
                    pR, pZ, pN, pG = get_banks(t)
                    for dst, mlo in ((pR, 0), (pN, 8), (pZ, 4)):
                        for mi in range(4):
                            m = mlo + mi
                            for k in range(KT):
                                mm(dst[:, mi * B:(mi + 1) * B], wht(m, k),
                                   h_prev[:, k * B:(k + 1) * B], start=False,
                                   stop=mi == 3 and k == KT - 1)

                def emit_dummies(t, rt):
                    nd = N_DUMMY.get(lvl, 0)
                    if not nd:
                        return
                    w = 4 * B
                    junk = gp[t & 1].tile([P, 512], FP, tag=f"g{t & 1}",
                                          name=f"junk{sfx}")
                    for i in range(nd):
                        mm(junk[:, :w], wht(i % MT, i % KT), rt[:, :w],
                           start=True, stop=True)

                h_prev = h0_tile if lvl != 3 else hz
                emit_prep(0, final=lvl == 3)
                for t in range(T):
                    pR, pZ, pN, pG = get_banks(t)
                    if lvl != 3 or t > 0:
                        emit_gh(t, h_prev)
                    if t + 1 < T:
                        emit_prep(t + 1, final=False)
                    # ---- eltwise tail ----
                    rt = wpool.tile([P, 4 * B], BF, tag=f"rt{sfx}")
                    nc.scalar.activation(rt[:, :], pR[:, :4 * B], AF.Sigmoid)
                    emit_dummies(t, rt)
                    bnw = wpool.tile([P, 4 * B], BF, tag=f"bn{sfx}")
                    nc.vector.tensor_mul(bnw[:, :], rt[:, :], pN[:, :4 * B])
                    ct = wpool.tile([P, 4 * B], FP, tag=f"ct{sfx}")
                    if lvl == 3:
                        nc.vector.tensor_add(ct[:, :], bnw[:, :],
                                             pG[:, :4 * B])
                    else:
                        nc.vector.tensor_add(ct[:, :], bnw[:, :],
                                             gin_sb_of_t(t)[:, 8 * B:12 * B])
                    nt = wpool.tile([P, 4 * B], BF, tag=f"nt{sfx}")
                    nc.scalar.activation(nt[:, :], ct[:, :], AF.Tanh)
                    et = wpool.tile([P, 4 * B], BF, tag=f"et{sfx}")
                    nc.vector.tensor_sub(et[:, :], h_prev[:, :], nt[:, :])
                    zt = wpool.tile([P, 4 * B], BF, tag=f"zt{sfx}")
                    nc.scalar.activation(zt[:, :], pZ[:, :4 * B], AF.Sigmoid)
                    ft = wpool.tile([P, 4 * B], BF, tag=f"ft{sfx}")
                    nc.vector.tensor_mul(ft[:, :], zt[:, :], et[:, :])
                    h_new = hA if t & 1 == 0 else hB
                    nc.vector.tensor_add(h_new[:, :], nt[:, :], ft[:, :])
                    if t == 0:
                        nc.gpsimd.tensor_copy(acc[:, :], h_new[:, :])
                    else:
                        nc.gpsimd.tensor_add(acc[:, :], acc[:, :], h_new[:, :])
                    h_prev = h_new

                return h_prev, acc

            # ================= LEVEL 3 =================
            xtv = xt.rearrange("p (t k b) -> p t k b", t=T, k=KT)

            def x3_rhs(t, k):
                return xtv[:, t, k]

            h3, acc3 = run_level(
                3, NB, None, x3_rhs, None,
                br64[:, :], bz64[:, :], bg64[:, :])

            # ---------------- Level 3 -> 2 transition ----------------
            # x2[p, k, t, j] = acc3[p, k, j, t] / 8   (j = parent)
            x2 = spool.tile([P, KT * NB], BF, tag="x2")
            x2v = x2[:].rearrange("p (k t j) -> p k t j", k=KT, t=A)
            acc3p = acc3[:].rearrange("p (k j t) -> p k t j", k=KT, j=A)
            nc.scalar.mul(x2v, acc3p, 1.0 / A)
            # h0_2 = mean over children of h3
            hr2 = spool.tile([P, KT * A], FP, tag="hr2")
            nc.vector.tensor_reduce(
                hr2[:].rearrange("p (k j) -> p k j", k=KT),
                h3[:].rearrange("p (k j t) -> p k j t", k=KT, j=A),
                axis=mybir.AxisListType.X, op=OP.add)
            h02 = spool.tile([P, KT * A], BF, tag="h02")
            nc.scalar.mul(h02[:, :], hr2[:, :], 1.0 / A)

            # batched gi2: two temp banks, m0-5 and m6-11
            giA = pg0.tile([P, 512], FP, tag="g0")
            giB = pg1.tile([P, 512], FP, tag="g1")
            for mg, bank in ((0, giA), (6, giB)):
                for mi in range(6):
                    m = mg + mi
                    for k in range(KT):
                        mm(bank[:, mi * NB:(mi + 1) * NB], wit(m, k),
                           x2[:, k * NB:(k + 1) * NB],
                           start=mi == 0 and k == 0,
                           stop=mi == 5 and k == KT - 1)
            # drain + bias into SBUF, layout [t][(m,b)]
            gi2b = spool.tile([P, T * MT * A], BF, tag="gi2sb")
            gi2bv = gi2b[:].rearrange("p (t m b) -> p m t b", t=T, m=MT)
            bb2v = bb2.rearrange("p (t m b) -> p m t b", t=T, m=MT)
            nc.vector.tensor_add(
                gi2bv[:, 0:6],
                giA[:, :6 * NB].rearrange("p (m t b) -> p m t b", m=6, t=T),
                bb2v[:, 0:6])
            nc.vector.tensor_add(
                gi2bv[:, 6:12],
                giB[:, :6 * NB].rearrange("p (m t b) -> p m t b", m=6, t=T),
                bb2v[:, 6:12])

            def gin2(t):
                return gi2b[:, t * MT * A:(t + 1) * MT * A]

            h2, acc2 = run_level(2, A, h02, None, gin2, None, None, None)

            # ---------------- Level 2 -> 1 transition ----------------
            x1 = spool.tile([P, KT * A], BF, tag="x1")
            nc.scalar.mul(x1[:, :], acc2[:, :], 1.0 / A)
            hr1 = spool.tile([P, KT], FP, tag="hr1")
            nc.vector.tensor_reduce(
                hr1[:].rearrange("p (k j) -> p k j", k=KT),
                h2[:].rearrange("p (k j t) -> p k j t", k=KT, j=1),
                axis=mybir.AxisListType.X, op=OP.add)
            h01 = spool.tile([P, KT], BF, tag="h01")
            nc.scalar.mul(h01[:, :], hr1[:, :], 1.0 / A)

            gi1t = pg0.tile([P, 512], FP, tag="g0")
            for m in range(MT):
                for k in range(KT):
                    mm(gi1t[:, m * A:(m + 1) * A], wit(m, k),
                       x1[:, k * A:(k + 1) * A],
                       start=m == 0 and k == 0,
                       stop=m == MT - 1 and k == KT - 1)
            gi1b = spool.tile([P, T * MT], BF, tag="gi1sb")
            nc.vector.tensor_add(
                gi1b[:].rearrange("p (t m) -> p m t", t=T),
                gi1t[:, :MT * A].rearrange("p (m t) -> p m t", m=MT),
                bb1.rearrange("p (t m) -> p m t", t=T))

            def gin1(t):
                return gi1b[:, t * MT:(t + 1) * MT]

            h1, acc1 = run_level(1, 1, h01, None, gin1, None, None, None)

            out_sb = spool.tile([P, KT], FP, tag="outsb")
            nc.scalar.mul(out_sb[:, :], acc1[:, :], 1.0 / A)
            nc.sync.dma_start(out=outp[:, :], in_=out_sb[:, :])

    nc.finalize()
    return nc


def _get_nc():
    global _BUILT
    if _BUILT is None:
        _BUILT = _build_nc()
    return _BUILT


def make_inputs(leaf_ids, embed_table, W_ih, W_hh, b_ih, b_hh):
    """Host-side layout prep: gather looked-up embedding rows and lay all
    tensors out in the on-chip transposed format (one bf16 blob)."""
    import ml_dtypes

    leaf_ids = np.asarray(leaf_ids).astype(np.int64)
    emb = np.asarray(embed_table, dtype=np.float32)
    W_ih = np.asarray(W_ih, dtype=np.float32)
    W_hh = np.asarray(W_hh, dtype=np.float32)
    b_ih = np.asarray(b_ih, dtype=np.float32)
    b_hh = np.asarray(b_hh, dtype=np.float32)

    x = emb[leaf_ids]                              # [node(64), t(8), 512]
    # xt[p, t, k, node]
    xt = np.ascontiguousarray(
        x.reshape(NB, T, KT, P).transpose(3, 1, 2, 0)).reshape(P, -1)

    def pack_w(W):  # [1536, 512] -> [P, (m,k) tiles]
        WT = np.ascontiguousarray(W.T)             # [512, 1536]
        return np.ascontiguousarray(
            WT.reshape(KT, P, MT, P).transpose(1, 2, 0, 3)).reshape(P, -1)

    ident = np.eye(P, dtype=np.float32)

    gb12 = np.concatenate([(b_ih + b_hh)[:2 * D], b_ih[2 * D:]]).reshape(MT, P)
    bhn4 = b_hh[2 * D:].reshape(KT, P)

    def bcast(rows, b):                            # rows [4, P] -> [P, 4*b]
        return np.repeat(rows.T[:, :, None], b, axis=2).reshape(P, -1)

    br64 = bcast(gb12[0:4], NB)
    bz64 = bcast(gb12[4:8], NB)
    bg64 = bcast(gb12[8:12], NB)
    bn64 = bcast(bhn4, NB)
    bn8 = bcast(bhn4, A)
    bn1 = bcast(bhn4, 1)
    # bb2[p, t, m, b] = gb12[m, p]
    bb2 = np.tile(gb12.T[:, None, :, None], (1, T, 1, A)).reshape(P, -1)
    bb1 = np.tile(gb12.T[:, None, :, None], (1, T, 1, 1)).reshape(P, -1)

    blob16 = np.concatenate(
        [ident, br64, bz64, bg64, bn64, bn8, bn1, bb2, bb1,
         xt, pack_w(W_ih), pack_w(W_hh)], axis=1).astype(ml_dtypes.bfloat16)
    assert blob16.shape == (P, B16_COLS), blob16.shape
    return {"blob16": np.ascontiguousarray(blob16)}


def unpack_output(out_np):
    # out [P, KT]: element (p, k) = root dim k*128+p
    return np.ascontiguousarray(out_np.T).reshape(1, 1, D).astype(np.float32)


def kernel(leaf_ids=None, layer=None, embed_table=None, W_ih=None, W_hh=None,
           b_ih=None, b_hh=None, **_unused):
    in_map = make_inputs(leaf_ids, embed_table, W_ih, W_hh, b_ih, b_hh)
    nc = _get_nc()
    res = run_bass_kernel_spmd(nc, [in_map] * N_CORES, list(range(N_CORES)))
    return unpack_output(res.results[0]["out"])
